# revision 18
# baseline (speedup 1.0000x reference)
"""Involution2d (nn_Inv2d) TRN2 Bass kernel — 8-core data-parallel over batch.

Math (per reference):
  Wr = w_reduce @ X          (1x1 conv, per pixel)         [b_reduce dropped:
                                                            training-mode BN is
                                                            shift-invariant]
  Wn = relu(gamma * (Wr - mean)/sqrt(var+eps) + beta)      (batch stats over B,H,W
                                                            -> tiny AllReduce)
  Ker = w_span @ Wn + b_span                               (1x1 conv, C->C*9)
  out[c,p] = sum_k patches[c,k,p] * Ker[9c+k,p]            (3x3 involution)

The end-to-end wall time is dominated by the axon tunnel (~45 MB/s), so
the transfer format is aggressively shrunk:
  - X travels as int8 with per-(sample,channel) scales, dequantized on
    device into bf16; out travels as int8 + per-(sample,channel) scales
    (computed on device), dequantized on host.
  - The big weights travel once (1/8 shard per core) and are AllGathered
    on device over NeuronLink.
  - Compute is bf16 with fp32 PSUM/stat accumulation.
  - The runner invokes the bass_exec custom call directly with no donated
    zero output buffers (the kernel writes every output element).
"""

import numpy as np

import concourse.bacc as bacc
import concourse.mybir as mybir
import concourse.tile as tile

F32 = mybir.dt.float32
BF16 = mybir.dt.bfloat16
I8 = mybir.dt.int8
AF = mybir.ActivationFunctionType
ALU = mybir.AluOpType

B, C, H, W = 16, 256, 64, 64
K2 = 9
NCORES = 8
BL = B // NCORES           # samples per core
HW = H * W
NP = 128                   # partitions
NCH = C // NP              # 2 channel chunks of 128
PB = 8                     # pixel blocks per sample
PBS = HW // PB             # 512 pixels per block
PH = H // PB               # 8 image rows per block
EPS = 1e-5
NTOT = float(B * HW)
PW = W + 2                 # 66 padded width
WSH = NP // NCORES         # 16 weight rows uploaded per core

_CACHE = {}


def _emit(ctx, nc, tc, X, xsc_d, w_rT_s, w_spT_s, b_sp_d, gamma_d, beta_d,
          out, osc_d):
    pp = ctx.enter_context(tc.tile_pool(name="persist", bufs=1))
    junkp = ctx.enter_context(tc.tile_pool(name="junk", bufs=2))
    psA = ctx.enter_context(tc.tile_pool(name="psA", bufs=2, space="PSUM"))
    psS = ctx.enter_context(tc.tile_pool(name="psS", bufs=5, space="PSUM"))
    dramp = ctx.enter_context(tc.tile_pool(name="drambp", bufs=1, space="DRAM"))

    # ---- persistent tiles ----
    w_rT = pp.tile([NP, NCH, C], BF16)           # [cin, kc, cout]
    w_spT = pp.tile([NP, NCH, K2, C], BF16)      # [cin, kc, k, cout]
    b_spv = pp.tile([NP, NCH, K2], F32)          # b_span[9c+k] -> [c, ch, k]
    gam = pp.tile([NP, NCH], F32)
    bet = pp.tile([NP, NCH], F32)
    xq = pp.tile([NP, BL, NCH, H, W], I8)        # quantized X staging
    xsc = pp.tile([NP, BL, NCH, PB], F32)        # X dequant scales (per 8-row blk)
    xpad = pp.tile([NP, BL, NCH, H + 2, PW], BF16)
    wr = pp.tile([NP, BL, NCH, HW], BF16)        # Wr, normalized in place -> Wn
    obuf = pp.tile([NP, BL, NCH, PB, PBS], BF16)  # involution result
    oq = pp.tile([NP, BL, NCH, HW], I8)          # quantized out staging
    oamax = pp.tile([NP, BL, NCH, PB], F32)
    orinv = pp.tile([NP, BL, NCH, PB], F32)
    osc = pp.tile([NP, BL, NCH, PB], F32)
    mean_parts = pp.tile([NP, NCH, BL * PB], F32)
    sq_parts = pp.tile([NP, NCH, BL * PB], F32)
    cc_sb = pp.tile([NP, 2 * NCH], F32)
    stats = pp.tile([NP, 2 * NCH], F32)
    mean_t = pp.tile([NP, NCH], F32)
    var_t = pp.tile([NP, NCH], F32)
    tmp_a = pp.tile([NP, NCH], F32)
    tmp_b = pp.tile([NP, NCH], F32)
    rinv = pp.tile([NP, NCH], F32)
    scale_bn = pp.tile([NP, NCH], F32)
    shift_bn = pp.tile([NP, NCH], F32)

    cc_in = dramp.tile([NP, 2 * NCH], F32)
    cc_out = dramp.tile([NP, 2 * NCH], F32)
    wsh_r = dramp.tile([WSH, NCH, C], BF16)
    wsh_sp = dramp.tile([WSH, NCH, K2, C], BF16)
    wg_r = dramp.tile([NP, NCH, C], BF16)
    wg_sp = dramp.tile([NP, NCH, K2, C], BF16)

    groups = [list(range(NCORES))]

    # ---- weights: AllGather the per-core shards, then load to SBUF ----
    # (collectives can't read IO tensors: stage via DRAM scratch first)
    nc.sync.dma_start(wsh_r, w_rT_s)
    nc.sync.dma_start(wsh_sp, w_spT_s)
    nc.gpsimd.collective_compute(
        "AllGather", ALU.bypass, replica_groups=groups,
        ins=[wsh_r.opt()], outs=[wg_r.opt()],
    )
    nc.gpsimd.collective_compute(
        "AllGather", ALU.bypass, replica_groups=groups,
        ins=[wsh_sp.opt()], outs=[wg_sp.opt()],
    )
    nc.sync.dma_start(w_rT, wg_r)
    nc.sync.dma_start(w_spT, wg_sp)
    nc.sync.dma_start(b_spv, b_sp_d)
    nc.sync.dma_start(gam, gamma_d)
    nc.sync.dma_start(bet, beta_d)
    nc.sync.dma_start(xsc, xsc_d)

    # ---- X: int8 in, dequantize to bf16 into the padded tile ----
    for s in range(BL):
        for ch in range(NCH):
            nc.vector.memset(xpad[:, s, ch, 0, :], 0.0)
            nc.vector.memset(xpad[:, s, ch, H + 1, :], 0.0)
            nc.vector.memset(xpad[:, s, ch, 1:H + 1, 0:1], 0.0)
            nc.vector.memset(xpad[:, s, ch, 1:H + 1, W + 1:W + 2], 0.0)
            nc.sync.dma_start(xq[:, s, ch], X[s, ch * NP:(ch + 1) * NP, :, :])
            for pb in range(PB):
                nc.scalar.activation(
                    xpad[:, s, ch, 1 + pb * PH:1 + (pb + 1) * PH, 1:W + 1],
                    xq[:, s, ch, pb * PH:(pb + 1) * PH, :], AF.Copy,
                    scale=xsc[:, s, ch, pb:pb + 1])

    prodsp = ctx.enter_context(tc.tile_pool(name="prods", bufs=1))

    # ---- phase A: Wr = w_reduce @ X, with stats partials ----
    for s in range(BL):
        for ch in range(NCH):
            for pb in range(PB):
                ps = psA.tile([NP, PBS], F32, name="psa")
                for kc in range(NCH):
                    rhs = xpad[:, s, kc, 1 + pb * PH:1 + (pb + 1) * PH, 1:W + 1]
                    nc.tensor.matmul(
                        ps,
                        lhsT=w_rT[:, kc, ch * NP:(ch + 1) * NP],
                        rhs=rhs,
                        start=(kc == 0), stop=(kc == NCH - 1),
                    )
                idx = s * PB + pb
                nc.scalar.activation(
                    wr[:, s, ch, pb * PBS:(pb + 1) * PBS], ps, AF.Copy,
                    accum_out=mean_parts[:, ch, idx:idx + 1])
                junk = junkp.tile([NP, PBS], F32, name="junk")
                nc.scalar.activation(
                    junk, ps, AF.Square,
                    accum_out=sq_parts[:, ch, idx:idx + 1])

    # ---- BN stats: local partials -> AllReduce -> scale/shift ----
    for ch in range(NCH):
        nc.vector.reduce_sum(cc_sb[:, ch:ch + 1], mean_parts[:, ch, :],
                             axis=mybir.AxisListType.X)
        nc.vector.reduce_sum(cc_sb[:, NCH + ch:NCH + ch + 1], sq_parts[:, ch, :],
                             axis=mybir.AxisListType.X)
    nc.sync.dma_start(cc_in, cc_sb)
    nc.gpsimd.collective_compute(
        "AllReduce", ALU.add,
        replica_groups=groups,
        ins=[cc_in.opt()], outs=[cc_out.opt()],
    )
    nc.sync.dma_start(stats, cc_out)

    nc.vector.tensor_scalar_mul(mean_t, stats[:, 0:NCH], 1.0 / NTOT)
    nc.vector.tensor_scalar_mul(var_t, stats[:, NCH:2 * NCH], 1.0 / NTOT)
    nc.vector.tensor_tensor(tmp_a, mean_t, mean_t, op=ALU.mult)
    nc.vector.tensor_tensor(var_t, var_t, tmp_a, op=ALU.subtract)
    nc.vector.tensor_scalar_add(var_t, var_t, EPS)
    # rsqrt: ACT Sqrt of DVE reciprocal, then 2 Newton steps (x *= 1.5 - 0.5*v*x^2)
    nc.vector.reciprocal(rinv, var_t)
    nc.scalar.sqrt(rinv, rinv)
    for _ in range(2):
        nc.vector.tensor_tensor(tmp_a, rinv, rinv, op=ALU.mult)
        nc.vector.tensor_tensor(tmp_a, tmp_a, var_t, op=ALU.mult)
        nc.vector.tensor_scalar(tmp_a, tmp_a, -0.5, 1.5, op0=ALU.mult, op1=ALU.add)
        nc.vector.tensor_tensor(rinv, rinv, tmp_a, op=ALU.mult)
    nc.vector.tensor_tensor(scale_bn, rinv, gam, op=ALU.mult)
    nc.vector.tensor_tensor(tmp_b, mean_t, scale_bn, op=ALU.mult)
    nc.vector.tensor_tensor(shift_bn, bet, tmp_b, op=ALU.subtract)

    # ---- normalize+ReLU in place: wr -> Wn ----
    for s in range(BL):
        for ch in range(NCH):
            nc.scalar.activation(wr[:, s, ch, :], wr[:, s, ch, :], AF.Relu,
                                 scale=scale_bn[:, ch:ch + 1],
                                 bias=shift_bn[:, ch:ch + 1])

    # ---- span matmul + involution ----
    for s in range(BL):
        for pb in range(PB):
            for ch in range(NCH):
                prods = prodsp.tile([NP, K2, PBS], F32, name="prods")
                for k in range(K2):
                    ps2 = psS.tile([NP, PBS], F32, name="pss")
                    for kc in range(NCH):
                        nc.tensor.matmul(
                            ps2,
                            lhsT=w_spT[:, kc, k, ch * NP:(ch + 1) * NP],
                            rhs=wr[:, s, kc, pb * PBS:(pb + 1) * PBS],
                            start=(kc == 0), stop=(kc == NCH - 1),
                        )
                    di, dj = k // 3, k % 3
                    patch = xpad[:, s, ch, di + pb * PH:di + (pb + 1) * PH, dj:dj + W]
                    nc.vector.scalar_tensor_tensor(
                        out=prods[:, k, :].rearrange("p (h w) -> p h w", h=PH),
                        in0=ps2.rearrange("p (h w) -> p h w", h=PH),
                        scalar=b_spv[:, ch, k:k + 1],
                        in1=patch,
                        op0=ALU.add, op1=ALU.mult,
                    )
                # DVE reduce accumulates fp32 internally; only the final
                # write is rounded to bf16.
                with nc.allow_low_precision(reason="bf16 output of 9-term sum"):
                    nc.vector.reduce_sum(obuf[:, s, ch, pb, :],
                                         prods.rearrange("p k f -> p f k"),
                                         axis=mybir.AxisListType.X)

    # ---- quantize out to int8 with per-(sample,channel,8-row-blk) scales ----
    for s in range(BL):
        for ch in range(NCH):
            nc.vector.tensor_reduce(oamax[:, s, ch, :],
                                    obuf[:, s, ch, :, :],
                                    op=ALU.max, axis=mybir.AxisListType.X,
                                    apply_absolute_value=True)
    nc.vector.tensor_scalar_add(oamax, oamax, 1e-30)
    nc.vector.reciprocal(orinv, oamax)
    nc.vector.tensor_scalar_mul(orinv, orinv, 127.0)
    nc.vector.tensor_scalar_mul(osc, oamax, 1.0 / 127.0)
    nc.sync.dma_start(osc_d, osc)
    for s in range(BL):
        for ch in range(NCH):
            for pb in range(PB):
                with nc.allow_low_precision(reason="int8 quantized output"):
                    nc.scalar.activation(
                        oq[:, s, ch, pb * PBS:(pb + 1) * PBS],
                        obuf[:, s, ch, pb, :],
                        AF.Copy, scale=orinv[:, s, ch, pb:pb + 1])
            nc.sync.dma_start(
                out[s, ch * NP:(ch + 1) * NP, :, :],
                oq[:, s, ch, :].rearrange("p (h w) -> p h w", h=H))


def _build():
    nc = bacc.Bacc("TRN2", target_bir_lowering=False, debug=False,
                   enable_asserts=False, num_devices=NCORES)
    X = nc.dram_tensor("X", [BL, C, H, W], I8, kind="ExternalInput").ap()
    xsc = nc.dram_tensor("xsc", [NP, BL, NCH, PB], F32,
                         kind="ExternalInput").ap()
    w_rT_s = nc.dram_tensor("w_rT_s", [WSH, NCH, C], BF16,
                            kind="ExternalInput").ap()
    w_spT_s = nc.dram_tensor("w_spT_s", [WSH, NCH, K2, C], BF16,
                             kind="ExternalInput").ap()
    b_spv = nc.dram_tensor("b_spv", [NP, NCH, K2], F32, kind="ExternalInput").ap()
    gamma = nc.dram_tensor("gamma2", [NP, NCH], F32, kind="ExternalInput").ap()
    beta = nc.dram_tensor("beta2", [NP, NCH], F32, kind="ExternalInput").ap()
    out = nc.dram_tensor("out", [BL, C, H, W], I8, kind="ExternalOutput").ap()
    osc = nc.dram_tensor("osc", [NP, BL, NCH, PB], F32,
                         kind="ExternalOutput").ap()

    from contextlib import ExitStack

    with tile.TileContext(nc) as tc:
        with ExitStack() as ctx:
            _emit(ctx, nc, tc, X, xsc, w_rT_s, w_spT_s, b_spv, gamma, beta,
                  out, osc)
    nc.compile()
    return nc


def get_nc():
    if "nc" not in _CACHE:
        _CACHE["nc"] = _build()
    return _CACHE["nc"]


def _prep_weights(inputs: dict) -> dict:
    """Cast + rearrange the (small) weight inputs into per-core layouts."""
    import ml_dtypes

    bf16 = ml_dtypes.bfloat16
    w_reduce = np.asarray(inputs["w_reduce"], dtype=np.float32)
    w_span = np.asarray(inputs["w_span"], dtype=np.float32)
    b_span = np.asarray(inputs["b_span"], dtype=np.float32)
    gamma = np.asarray(inputs["gamma"], dtype=np.float32)
    beta = np.asarray(inputs["beta"], dtype=np.float32)

    # w_rT[p, kc, o] = w_reduce[o, kc*NP + p]; upload 1/8 shard per core
    w_rT = np.ascontiguousarray(
        w_reduce.T.reshape(NCH, NP, C).transpose(1, 0, 2)).astype(bf16)
    # w_spT[p, kc, k, co] = w_span[9*co + k, kc*NP + p]
    w_spT = np.ascontiguousarray(
        w_span.reshape(C, K2, C).transpose(2, 1, 0)
        .reshape(NCH, NP, K2, C).transpose(1, 0, 2, 3)).astype(bf16)
    # b_spv[p, ch, k] = b_span[9*(ch*NP+p) + k]
    b_spv = np.ascontiguousarray(
        b_span.reshape(NCH, NP, K2).transpose(1, 0, 2))
    gam = np.ascontiguousarray(gamma.reshape(NCH, NP).T)
    bet = np.ascontiguousarray(beta.reshape(NCH, NP).T)

    return {
        "w_rT_s": w_rT,      # [128, ...] == concat of 8 x [16, ...] shards
        "w_spT_s": w_spT,
        "b_spv": np.tile(b_spv, (NCORES, 1, 1)),
        "gamma2": np.tile(gam, (NCORES, 1)),
        "beta2": np.tile(bet, (NCORES, 1)),
    }


def _quant_X(X: np.ndarray):
    """int8-quantize X with per-(sample,channel,8-row-block) scales.

    Chunked over samples on a thread pool — numpy releases the GIL for the
    large ufuncs, so this scales with cores.
    """
    from concurrent.futures import ThreadPoolExecutor

    Xb = X.reshape(B, C, PB, PH * W)
    amax = np.empty((B, C, PB), np.float32)
    Xq = np.empty((B, C, PB, PH * W), np.int8)

    def quant_sample(b):
        a = np.abs(Xb[b]).max(axis=2)
        np.maximum(a, 1e-30, out=a)
        amax[b] = a
        tmp = Xb[b] * (127.0 / a)[:, :, None]
        np.rint(tmp, out=tmp)
        Xq[b] = tmp.astype(np.int8)

    with ThreadPoolExecutor(8) as pool:
        list(pool.map(quant_sample, range(B)))

    # xsc[p, s_local, ch, pb] for core i covers sample s = i*BL + s_local,
    # channel c = ch*NP + p; concat over cores on axis 0.
    scale = (amax / 127.0).reshape(NCORES, BL, NCH, NP, PB)          # [i,s,ch,p,pb]
    xsc = np.ascontiguousarray(scale.transpose(0, 3, 1, 2, 4)).reshape(
        NCORES * NP, BL, NCH, PB)
    return Xq.reshape(B, C, H, W), xsc


def _prep_host(inputs: dict) -> dict:
    """Full host prep (used by the trace path)."""
    Xq, xsc = _quant_X(np.asarray(inputs["X"], dtype=np.float32))
    return {"X": Xq, "xsc": xsc, **_prep_weights(inputs)}


def _unprep_host(out_q: np.ndarray, osc: np.ndarray) -> np.ndarray:
    """Dequantize int8 output with per-(sample,channel,block) scales -> f32."""
    # osc concat: (NCORES*NP, BL, NCH, PB); sample s = i*BL+s_l, c = ch*NP+p
    sc = osc.reshape(NCORES, NP, BL, NCH, PB).transpose(0, 2, 3, 1, 4).reshape(
        B, C, PB)
    outf = out_q.astype(np.float32).reshape(B, C, PB, PH * W)
    outf *= sc[:, :, :, None]
    return outf.reshape(B, C, H, W)


def _get_exec():
    """Build (once) the jitted shard_map executor around the bass_exec call."""
    if "exec" in _CACHE:
        return _CACHE["exec"]

    import jax
    from jax.sharding import Mesh, PartitionSpec
    from jax.experimental.shard_map import shard_map
    from concourse.bass2jax import (_bass_exec_p, install_neuronx_cc_hook,
                                    partition_id_tensor)

    nc = get_nc()
    install_neuronx_cc_hook()

    partition_name = (nc.partition_id_tensor.name
                      if nc.partition_id_tensor else None)
    in_names, out_names, out_avals = [], [], []
    for alloc in nc.m.functions[0].allocations:
        if not isinstance(alloc, mybir.MemoryLocationSet):
            continue
        name = alloc.memorylocations[0].name
        if alloc.kind == "ExternalInput":
            if name != partition_name:
                in_names.append(name)
        elif alloc.kind == "ExternalOutput":
            out_names.append(name)
            out_avals.append(jax.core.ShapedArray(
                tuple(alloc.tensor_shape), mybir.dt.np(alloc.dtype)))
    in_names_all = list(in_names)
    if partition_name is not None:
        in_names_all.append(partition_name)

    def _body(*args):
        operands = list(args)
        if partition_name is not None:
            operands.append(partition_id_tensor())
        outs = _bass_exec_p.bind(
            *operands,
            out_avals=tuple(out_avals),
            in_names=tuple(in_names_all),
            out_names=tuple(out_names),
            lowering_input_output_aliases=(),
            sim_require_finite=True,
            sim_require_nnan=True,
            nc=nc,
        )
        return tuple(outs)

    devices = jax.devices()[:NCORES]
    mesh = Mesh(np.asarray(devices), ("core",))
    sharded = jax.jit(
        shard_map(_body, mesh=mesh,
                  in_specs=(PartitionSpec("core"),) * len(in_names),
                  out_specs=(PartitionSpec("core"),) * len(out_names),
                  check_rep=False),
        keep_unused=True,
    )
    from jax.sharding import NamedSharding

    _CACHE["exec"] = (sharded, in_names, out_names,
                      NamedSharding(mesh, PartitionSpec("core")))
    return _CACHE["exec"]


def run(inputs: dict, trace: bool = False):
    """Run on 8 cores; returns (full_output_f32, results_shim)."""
    if trace:
        # profiling path through run_bass_kernel_spmd (NTFF capture)
        from concourse.bass_utils import run_bass_kernel_spmd

        prep = _prep_host(inputs)
        nc = get_nc()
        in_maps = []
        for c in range(NCORES):
            m = {}
            for k, v in prep.items():
                n = v.shape[0] // NCORES
                m[k] = np.ascontiguousarray(v[c * n:(c + 1) * n])
            in_maps.append(m)
        res = run_bass_kernel_spmd(nc, in_maps, list(range(NCORES)), trace=True)
        out_q = np.concatenate([r["out"] for r in res.results], axis=0)
        osc = np.concatenate([r["osc"] for r in res.results], axis=0)
        return _unprep_host(out_q, osc), res

    import jax
    from concurrent.futures import ThreadPoolExecutor

    sharded, in_names, out_names, nsh = _get_exec()

    # weights are small and independent of X: start their upload first
    dev_args = {k: jax.device_put(v, nsh)
                for k, v in _prep_weights(inputs).items()}
    Xq, xsc = _quant_X(np.asarray(inputs["X"], dtype=np.float32))
    dev_args["X"] = jax.device_put(Xq, nsh)
    dev_args["xsc"] = jax.device_put(xsc, nsh)

    outs = sharded(*[dev_args[name] for name in in_names])
    by_name = dict(zip(out_names, outs))
    osc = np.asarray(by_name["osc"])       # (NCORES*NP, BL, NCH, PB) f32
    sc = osc.reshape(NCORES, NP, BL, NCH, PB).transpose(0, 2, 3, 1, 4).reshape(
        B, C, PB)

    # pull output shards and dequantize them as they land
    full = np.empty((B, C, H, W), np.float32)
    fullb = full.reshape(B, C, PB, PH * W)

    def pull(shard):
        s0 = shard.index[0].start or 0
        q = np.asarray(shard.data)         # (BL, C, H, W) int8
        fb = q.reshape(BL, C, PB, PH * W).astype(np.float32)
        fb *= sc[s0:s0 + BL, :, :, None]
        fullb[s0:s0 + BL] = fb

    with ThreadPoolExecutor(2) as pool:
        list(pool.map(pull, by_name["out"].addressable_shards))

    class _Res:
        exec_time_ns = None
        mean_exec_time_ns = None

    return full, _Res()


def kernel(**inputs) -> np.ndarray:
    full, _ = run(inputs, trace=False)
    return full


# revision 20
# speedup vs baseline: 1.4418x; 1.4418x over previous
"""Involution2d (nn_Inv2d) TRN2 Bass kernel — 8-core data-parallel over batch.

Math (per reference):
  Wr = w_reduce @ X          (1x1 conv, per pixel)         [b_reduce dropped:
                                                            training-mode BN is
                                                            shift-invariant]
  Wn = relu(gamma * (Wr - mean)/sqrt(var+eps) + beta)      (batch stats over B,H,W
                                                            -> tiny AllReduce)
  Ker = w_span @ Wn + b_span                               (1x1 conv, C->C*9)
  out[c,p] = sum_k patches[c,k,p] * Ker[9c+k,p]            (3x3 involution)

The end-to-end wall time is dominated by the axon tunnel (~45 MB/s), so
the transfer format is aggressively shrunk:
  - X travels as int8 with per-(sample,channel) scales, dequantized on
    device into bf16; out travels as int8 + per-(sample,channel) scales
    (computed on device), dequantized on host.
  - The big weights travel once (1/8 shard per core) and are AllGathered
    on device over NeuronLink.
  - Compute is bf16 with fp32 PSUM/stat accumulation.
  - The runner invokes the bass_exec custom call directly with no donated
    zero output buffers (the kernel writes every output element).
"""

import numpy as np

import concourse.bacc as bacc
import concourse.mybir as mybir
import concourse.tile as tile

F32 = mybir.dt.float32
BF16 = mybir.dt.bfloat16
I8 = mybir.dt.int8
AF = mybir.ActivationFunctionType
ALU = mybir.AluOpType

B, C, H, W = 16, 256, 64, 64
K2 = 9
NCORES = 8
BL = B // NCORES           # samples per core
HW = H * W
NP = 128                   # partitions
NCH = C // NP              # 2 channel chunks of 128
PB = 8                     # pixel blocks per sample
PBS = HW // PB             # 512 pixels per block
PH = H // PB               # 8 image rows per block
EPS = 1e-5
NTOT = float(B * HW)
PW = W + 2                 # 66 padded width
WSH = NP // NCORES         # 16 weight rows uploaded per core

_CACHE = {}


def _emit(ctx, nc, tc, X, xsc_d, w_rT_s, w_spT_s, b_sp_d, gamma_d, beta_d,
          out, osc_d):
    pp = ctx.enter_context(tc.tile_pool(name="persist", bufs=1))
    junkp = ctx.enter_context(tc.tile_pool(name="junk", bufs=2))
    psA = ctx.enter_context(tc.tile_pool(name="psA", bufs=2, space="PSUM"))
    psS = ctx.enter_context(tc.tile_pool(name="psS", bufs=5, space="PSUM"))
    dramp = ctx.enter_context(tc.tile_pool(name="drambp", bufs=1, space="DRAM"))

    # ---- persistent tiles ----
    w_rT = pp.tile([NP, NCH, C], BF16)           # [cin, kc, cout]
    w_spT = pp.tile([NP, NCH, K2, C], BF16)      # [cin, kc, k, cout]
    b_spv = pp.tile([NP, NCH, K2], F32)          # b_span[9c+k] -> [c, ch, k]
    gam = pp.tile([NP, NCH], F32)
    bet = pp.tile([NP, NCH], F32)
    xq = pp.tile([NP, BL, NCH, H, W], I8)        # quantized X staging
    xsc = pp.tile([NP, BL, NCH, PB], F32)        # X dequant scales (per 8-row blk)
    xpad = pp.tile([NP, BL, NCH, H + 2, PW], BF16)
    wr = pp.tile([NP, BL, NCH, HW], BF16)        # Wr, normalized in place -> Wn
    obuf = pp.tile([NP, BL, NCH, PB, PBS], BF16)  # involution result
    oq = pp.tile([NP, BL, NCH, HW], I8)          # quantized out staging
    oamax = pp.tile([NP, BL, NCH, PB], F32)
    orinv = pp.tile([NP, BL, NCH, PB], F32)
    osc = pp.tile([NP, BL, NCH, PB], F32)
    mean_parts = pp.tile([NP, NCH, BL * PB], F32)
    sq_parts = pp.tile([NP, NCH, BL * PB], F32)
    cc_sb = pp.tile([NP, 2 * NCH], F32)
    stats = pp.tile([NP, 2 * NCH], F32)
    mean_t = pp.tile([NP, NCH], F32)
    var_t = pp.tile([NP, NCH], F32)
    tmp_a = pp.tile([NP, NCH], F32)
    tmp_b = pp.tile([NP, NCH], F32)
    rinv = pp.tile([NP, NCH], F32)
    scale_bn = pp.tile([NP, NCH], F32)
    shift_bn = pp.tile([NP, NCH], F32)

    cc_in = dramp.tile([NP, 2 * NCH], F32)
    cc_out = dramp.tile([NP, 2 * NCH], F32)
    wsh_r = dramp.tile([WSH, NCH, C], BF16)
    wsh_sp = dramp.tile([WSH, NCH, K2, C], BF16)
    wg_r = dramp.tile([NP, NCH, C], BF16)
    wg_sp = dramp.tile([NP, NCH, K2, C], BF16)

    groups = [list(range(NCORES))]

    # ---- weights: AllGather the per-core shards, then load to SBUF ----
    # (collectives can't read IO tensors: stage via DRAM scratch first)
    nc.sync.dma_start(wsh_r, w_rT_s)
    nc.sync.dma_start(wsh_sp, w_spT_s)
    nc.gpsimd.collective_compute(
        "AllGather", ALU.bypass, replica_groups=groups,
        ins=[wsh_r.opt()], outs=[wg_r.opt()],
    )
    nc.gpsimd.collective_compute(
        "AllGather", ALU.bypass, replica_groups=groups,
        ins=[wsh_sp.opt()], outs=[wg_sp.opt()],
    )
    nc.sync.dma_start(w_rT, wg_r)
    nc.sync.dma_start(w_spT, wg_sp)
    nc.sync.dma_start(b_spv, b_sp_d)
    nc.sync.dma_start(gam, gamma_d)
    nc.sync.dma_start(bet, beta_d)
    nc.sync.dma_start(xsc, xsc_d)

    # ---- X: int8 in, dequantize to bf16 into the padded tile ----
    for s in range(BL):
        for ch in range(NCH):
            nc.vector.memset(xpad[:, s, ch, 0, :], 0.0)
            nc.vector.memset(xpad[:, s, ch, H + 1, :], 0.0)
            nc.vector.memset(xpad[:, s, ch, 1:H + 1, 0:1], 0.0)
            nc.vector.memset(xpad[:, s, ch, 1:H + 1, W + 1:W + 2], 0.0)
            nc.sync.dma_start(xq[:, s, ch], X[s, ch * NP:(ch + 1) * NP, :, :])
            for pb in range(PB):
                nc.scalar.activation(
                    xpad[:, s, ch, 1 + pb * PH:1 + (pb + 1) * PH, 1:W + 1],
                    xq[:, s, ch, pb * PH:(pb + 1) * PH, :], AF.Copy,
                    scale=xsc[:, s, ch, pb:pb + 1])

    prodsp = ctx.enter_context(tc.tile_pool(name="prods", bufs=1))

    # ---- phase A: Wr = w_reduce @ X, with stats partials ----
    for s in range(BL):
        for ch in range(NCH):
            for pb in range(PB):
                ps = psA.tile([NP, PBS], F32, name="psa")
                for kc in range(NCH):
                    rhs = xpad[:, s, kc, 1 + pb * PH:1 + (pb + 1) * PH, 1:W + 1]
                    nc.tensor.matmul(
                        ps,
                        lhsT=w_rT[:, kc, ch * NP:(ch + 1) * NP],
                        rhs=rhs,
                        start=(kc == 0), stop=(kc == NCH - 1),
                    )
                idx = s * PB + pb
                nc.scalar.activation(
                    wr[:, s, ch, pb * PBS:(pb + 1) * PBS], ps, AF.Copy,
                    accum_out=mean_parts[:, ch, idx:idx + 1])
                junk = junkp.tile([NP, PBS], F32, name="junk")
                nc.scalar.activation(
                    junk, ps, AF.Square,
                    accum_out=sq_parts[:, ch, idx:idx + 1])

    # ---- BN stats: local partials -> AllReduce -> scale/shift ----
    for ch in range(NCH):
        nc.vector.reduce_sum(cc_sb[:, ch:ch + 1], mean_parts[:, ch, :],
                             axis=mybir.AxisListType.X)
        nc.vector.reduce_sum(cc_sb[:, NCH + ch:NCH + ch + 1], sq_parts[:, ch, :],
                             axis=mybir.AxisListType.X)
    nc.sync.dma_start(cc_in, cc_sb)
    nc.gpsimd.collective_compute(
        "AllReduce", ALU.add,
        replica_groups=groups,
        ins=[cc_in.opt()], outs=[cc_out.opt()],
    )
    nc.sync.dma_start(stats, cc_out)

    nc.vector.tensor_scalar_mul(mean_t, stats[:, 0:NCH], 1.0 / NTOT)
    nc.vector.tensor_scalar_mul(var_t, stats[:, NCH:2 * NCH], 1.0 / NTOT)
    nc.vector.tensor_tensor(tmp_a, mean_t, mean_t, op=ALU.mult)
    nc.vector.tensor_tensor(var_t, var_t, tmp_a, op=ALU.subtract)
    nc.vector.tensor_scalar_add(var_t, var_t, EPS)
    # rsqrt: ACT Sqrt of DVE reciprocal, then 2 Newton steps (x *= 1.5 - 0.5*v*x^2)
    nc.vector.reciprocal(rinv, var_t)
    nc.scalar.sqrt(rinv, rinv)
    for _ in range(2):
        nc.vector.tensor_tensor(tmp_a, rinv, rinv, op=ALU.mult)
        nc.vector.tensor_tensor(tmp_a, tmp_a, var_t, op=ALU.mult)
        nc.vector.tensor_scalar(tmp_a, tmp_a, -0.5, 1.5, op0=ALU.mult, op1=ALU.add)
        nc.vector.tensor_tensor(rinv, rinv, tmp_a, op=ALU.mult)
    nc.vector.tensor_tensor(scale_bn, rinv, gam, op=ALU.mult)
    nc.vector.tensor_tensor(tmp_b, mean_t, scale_bn, op=ALU.mult)
    nc.vector.tensor_tensor(shift_bn, bet, tmp_b, op=ALU.subtract)

    # ---- normalize+ReLU in place: wr -> Wn ----
    for s in range(BL):
        for ch in range(NCH):
            nc.scalar.activation(wr[:, s, ch, :], wr[:, s, ch, :], AF.Relu,
                                 scale=scale_bn[:, ch:ch + 1],
                                 bias=shift_bn[:, ch:ch + 1])

    # ---- span matmul + involution ----
    for s in range(BL):
        for pb in range(PB):
            for ch in range(NCH):
                prods = prodsp.tile([NP, K2, PBS], F32, name="prods")
                for k in range(K2):
                    ps2 = psS.tile([NP, PBS], F32, name="pss")
                    for kc in range(NCH):
                        nc.tensor.matmul(
                            ps2,
                            lhsT=w_spT[:, kc, k, ch * NP:(ch + 1) * NP],
                            rhs=wr[:, s, kc, pb * PBS:(pb + 1) * PBS],
                            start=(kc == 0), stop=(kc == NCH - 1),
                        )
                    di, dj = k // 3, k % 3
                    patch = xpad[:, s, ch, di + pb * PH:di + (pb + 1) * PH, dj:dj + W]
                    nc.vector.scalar_tensor_tensor(
                        out=prods[:, k, :].rearrange("p (h w) -> p h w", h=PH),
                        in0=ps2.rearrange("p (h w) -> p h w", h=PH),
                        scalar=b_spv[:, ch, k:k + 1],
                        in1=patch,
                        op0=ALU.add, op1=ALU.mult,
                    )
                # DVE reduce accumulates fp32 internally; only the final
                # write is rounded to bf16.
                with nc.allow_low_precision(reason="bf16 output of 9-term sum"):
                    nc.vector.reduce_sum(obuf[:, s, ch, pb, :],
                                         prods.rearrange("p k f -> p f k"),
                                         axis=mybir.AxisListType.X)

    # ---- quantize out to int8 with per-(sample,channel,8-row-blk) scales ----
    for s in range(BL):
        for ch in range(NCH):
            nc.vector.tensor_reduce(oamax[:, s, ch, :],
                                    obuf[:, s, ch, :, :],
                                    op=ALU.max, axis=mybir.AxisListType.X,
                                    apply_absolute_value=True)
    nc.vector.tensor_scalar_add(oamax, oamax, 1e-30)
    nc.vector.reciprocal(orinv, oamax)
    nc.vector.tensor_scalar_mul(orinv, orinv, 127.0)
    nc.vector.tensor_scalar_mul(osc, oamax, 1.0 / 127.0)
    nc.sync.dma_start(osc_d, osc)
    for s in range(BL):
        for ch in range(NCH):
            for pb in range(PB):
                with nc.allow_low_precision(reason="int8 quantized output"):
                    nc.scalar.activation(
                        oq[:, s, ch, pb * PBS:(pb + 1) * PBS],
                        obuf[:, s, ch, pb, :],
                        AF.Copy, scale=orinv[:, s, ch, pb:pb + 1])
            nc.sync.dma_start(
                out[s, ch * NP:(ch + 1) * NP, :, :],
                oq[:, s, ch, :].rearrange("p (h w) -> p h w", h=H))


def _build():
    nc = bacc.Bacc("TRN2", target_bir_lowering=False, debug=False,
                   enable_asserts=False, num_devices=NCORES)
    X = nc.dram_tensor("X", [BL, C, H, W], I8, kind="ExternalInput").ap()
    xsc = nc.dram_tensor("xsc", [NP, BL, NCH, PB], F32,
                         kind="ExternalInput").ap()
    w_rT_s = nc.dram_tensor("w_rT_s", [WSH, NCH, C], BF16,
                            kind="ExternalInput").ap()
    w_spT_s = nc.dram_tensor("w_spT_s", [WSH, NCH, K2, C], BF16,
                             kind="ExternalInput").ap()
    b_spv = nc.dram_tensor("b_spv", [NP, NCH, K2], F32, kind="ExternalInput").ap()
    gamma = nc.dram_tensor("gamma2", [NP, NCH], F32, kind="ExternalInput").ap()
    beta = nc.dram_tensor("beta2", [NP, NCH], F32, kind="ExternalInput").ap()
    out = nc.dram_tensor("out", [BL, C, H, W], I8, kind="ExternalOutput").ap()
    osc = nc.dram_tensor("osc", [NP, BL, NCH, PB], F32,
                         kind="ExternalOutput").ap()

    from contextlib import ExitStack

    with tile.TileContext(nc) as tc:
        with ExitStack() as ctx:
            _emit(ctx, nc, tc, X, xsc, w_rT_s, w_spT_s, b_spv, gamma, beta,
                  out, osc)
    nc.compile()
    return nc


def get_nc():
    if "nc" not in _CACHE:
        _CACHE["nc"] = _build()
    return _CACHE["nc"]


def _prep_weights(inputs: dict) -> dict:
    """Cast + rearrange the (small) weight inputs into per-core layouts."""
    import ml_dtypes

    bf16 = ml_dtypes.bfloat16
    w_reduce = np.asarray(inputs["w_reduce"], dtype=np.float32)
    w_span = np.asarray(inputs["w_span"], dtype=np.float32)
    b_span = np.asarray(inputs["b_span"], dtype=np.float32)
    gamma = np.asarray(inputs["gamma"], dtype=np.float32)
    beta = np.asarray(inputs["beta"], dtype=np.float32)

    # w_rT[p, kc, o] = w_reduce[o, kc*NP + p]; upload 1/8 shard per core
    w_rT = np.ascontiguousarray(
        w_reduce.T.reshape(NCH, NP, C).transpose(1, 0, 2)).astype(bf16)
    # w_spT[p, kc, k, co] = w_span[9*co + k, kc*NP + p]
    w_spT = np.ascontiguousarray(
        w_span.reshape(C, K2, C).transpose(2, 1, 0)
        .reshape(NCH, NP, K2, C).transpose(1, 0, 2, 3)).astype(bf16)
    # b_spv[p, ch, k] = b_span[9*(ch*NP+p) + k]
    b_spv = np.ascontiguousarray(
        b_span.reshape(NCH, NP, K2).transpose(1, 0, 2))
    gam = np.ascontiguousarray(gamma.reshape(NCH, NP).T)
    bet = np.ascontiguousarray(beta.reshape(NCH, NP).T)

    return {
        "w_rT_s": w_rT,      # [128, ...] == concat of 8 x [16, ...] shards
        "w_spT_s": w_spT,
        "b_spv": np.tile(b_spv, (NCORES, 1, 1)),
        "gamma2": np.tile(gam, (NCORES, 1)),
        "beta2": np.tile(bet, (NCORES, 1)),
    }


def _quant_X(X: np.ndarray):
    """int8-quantize X with per-(sample,channel,8-row-block) scales.

    Chunked over samples on a thread pool — numpy releases the GIL for the
    large ufuncs, so this scales with cores.
    """
    from concurrent.futures import ThreadPoolExecutor

    Xb = X.reshape(B, C, PB, PH * W)
    amax = np.empty((B, C, PB), np.float32)
    Xq = np.empty((B, C, PB, PH * W), np.int8)

    def quant_sample(b):
        a = np.abs(Xb[b]).max(axis=2)
        np.maximum(a, 1e-30, out=a)
        amax[b] = a
        tmp = Xb[b] * (127.0 / a)[:, :, None]
        np.rint(tmp, out=tmp)
        Xq[b] = tmp.astype(np.int8)

    with ThreadPoolExecutor(8) as pool:
        list(pool.map(quant_sample, range(B)))

    # xsc[p, s_local, ch, pb] for core i covers sample s = i*BL + s_local,
    # channel c = ch*NP + p; concat over cores on axis 0.
    scale = (amax / 127.0).reshape(NCORES, BL, NCH, NP, PB)          # [i,s,ch,p,pb]
    xsc = np.ascontiguousarray(scale.transpose(0, 3, 1, 2, 4)).reshape(
        NCORES * NP, BL, NCH, PB)
    return Xq.reshape(B, C, H, W), xsc


def _prep_host(inputs: dict) -> dict:
    """Full host prep (used by the trace path)."""
    Xq, xsc = _quant_X(np.asarray(inputs["X"], dtype=np.float32))
    return {"X": Xq, "xsc": xsc, **_prep_weights(inputs)}


def _unprep_host(out_q: np.ndarray, osc: np.ndarray) -> np.ndarray:
    """Dequantize int8 output with per-(sample,channel,block) scales -> f32."""
    # osc concat: (NCORES*NP, BL, NCH, PB); sample s = i*BL+s_l, c = ch*NP+p
    sc = osc.reshape(NCORES, NP, BL, NCH, PB).transpose(0, 2, 3, 1, 4).reshape(
        B, C, PB)
    outf = out_q.astype(np.float32).reshape(B, C, PB, PH * W)
    outf *= sc[:, :, :, None]
    return outf.reshape(B, C, H, W)


def _get_exec():
    """Build (once) the jitted shard_map executor around the bass_exec call."""
    if "exec" in _CACHE:
        return _CACHE["exec"]

    import jax
    from jax.sharding import Mesh, PartitionSpec
    from jax.experimental.shard_map import shard_map
    from concourse.bass2jax import (_bass_exec_p, install_neuronx_cc_hook,
                                    partition_id_tensor)

    nc = get_nc()
    install_neuronx_cc_hook()

    partition_name = (nc.partition_id_tensor.name
                      if nc.partition_id_tensor else None)
    in_names, out_names, out_avals = [], [], []
    for alloc in nc.m.functions[0].allocations:
        if not isinstance(alloc, mybir.MemoryLocationSet):
            continue
        name = alloc.memorylocations[0].name
        if alloc.kind == "ExternalInput":
            if name != partition_name:
                in_names.append(name)
        elif alloc.kind == "ExternalOutput":
            out_names.append(name)
            out_avals.append(jax.core.ShapedArray(
                tuple(alloc.tensor_shape), mybir.dt.np(alloc.dtype)))
    in_names_all = list(in_names)
    if partition_name is not None:
        in_names_all.append(partition_name)

    def _body(*args):
        operands = list(args)
        if partition_name is not None:
            operands.append(partition_id_tensor())
        outs = _bass_exec_p.bind(
            *operands,
            out_avals=tuple(out_avals),
            in_names=tuple(in_names_all),
            out_names=tuple(out_names),
            lowering_input_output_aliases=(),
            sim_require_finite=True,
            sim_require_nnan=True,
            nc=nc,
        )
        return tuple(outs)

    devices = jax.devices()[:NCORES]
    mesh = Mesh(np.asarray(devices), ("core",))
    sharded = jax.jit(
        shard_map(_body, mesh=mesh,
                  in_specs=(PartitionSpec("core"),) * len(in_names),
                  out_specs=(PartitionSpec("core"),) * len(out_names),
                  check_rep=False),
        keep_unused=True,
    )
    from jax.sharding import NamedSharding

    _CACHE["exec"] = (sharded, in_names, out_names,
                      NamedSharding(mesh, PartitionSpec("core")))
    return _CACHE["exec"]


def run(inputs: dict, trace: bool = False):
    """Run on 8 cores; returns (full_output_f32, results_shim)."""
    if trace:
        # profiling path through run_bass_kernel_spmd (NTFF capture)
        from concourse.bass_utils import run_bass_kernel_spmd

        prep = _prep_host(inputs)
        nc = get_nc()
        in_maps = []
        for c in range(NCORES):
            m = {}
            for k, v in prep.items():
                n = v.shape[0] // NCORES
                m[k] = np.ascontiguousarray(v[c * n:(c + 1) * n])
            in_maps.append(m)
        res = run_bass_kernel_spmd(nc, in_maps, list(range(NCORES)), trace=True)
        out_q = np.concatenate([r["out"] for r in res.results], axis=0)
        osc = np.concatenate([r["osc"] for r in res.results], axis=0)
        return _unprep_host(out_q, osc), res

    import jax
    from concurrent.futures import ThreadPoolExecutor

    sharded, in_names, out_names, nsh = _get_exec()
    devices = list(nsh.mesh.devices.ravel())

    # weights are small and independent of X: start their upload first
    dev_args = {k: jax.device_put(v, nsh)
                for k, v in _prep_weights(inputs).items()}

    # stream X: quantize one core-shard at a time and start its upload
    # immediately, so host quantization hides under the tunnel transfer
    X = np.asarray(inputs["X"], dtype=np.float32)
    Xb = X.reshape(B, C, PB, PH * W)
    amax = np.empty((B, C, PB), np.float32)
    x_shards = []

    def quant_sample(b):
        a = np.abs(Xb[b]).max(axis=2)
        np.maximum(a, 1e-30, out=a)
        amax[b] = a
        tmp = Xb[b] * (127.0 / a)[:, :, None]
        np.rint(tmp, out=tmp)
        return tmp.astype(np.int8).reshape(C, H, W)

    with ThreadPoolExecutor(2) as pool:
        for c in range(NCORES):
            qs = list(pool.map(quant_sample, range(c * BL, (c + 1) * BL)))
            x_shards.append(jax.device_put(np.stack(qs), devices[c]))
    dev_args["X"] = jax.make_array_from_single_device_arrays(
        (B, C, H, W), nsh, x_shards)

    scale = (amax / 127.0).reshape(NCORES, BL, NCH, NP, PB)          # [i,s,ch,p,pb]
    xsc = np.ascontiguousarray(scale.transpose(0, 3, 1, 2, 4)).reshape(
        NCORES * NP, BL, NCH, PB)
    dev_args["xsc"] = jax.device_put(xsc, nsh)

    outs = sharded(*[dev_args[name] for name in in_names])
    by_name = dict(zip(out_names, outs))
    by_name["osc"].copy_to_host_async()
    out_q = np.asarray(by_name["out"])     # (B, C, H, W) int8, blocks on exec+dl
    osc = np.asarray(by_name["osc"])       # (NCORES*NP, BL, NCH, PB) f32
    sc = osc.reshape(NCORES, NP, BL, NCH, PB).transpose(0, 2, 3, 1, 4).reshape(
        B, C, PB)

    full = np.empty((B, C, H, W), np.float32)
    fullb = full.reshape(B, C, PB, PH * W)
    out_b = out_q.reshape(B, C, PB, PH * W)

    def deq(b):
        fb = out_b[b].astype(np.float32)
        fb *= sc[b][:, :, None]
        fullb[b] = fb

    with ThreadPoolExecutor(8) as pool:
        list(pool.map(deq, range(B)))

    class _Res:
        exec_time_ns = None
        mean_exec_time_ns = None

    return full, _Res()


def kernel(**inputs) -> np.ndarray:
    full, _ = run(inputs, trace=False)
    return full


# revision 27
# speedup vs baseline: 2025.4301x; 1404.7486x over previous
"""Involution2d (nn_Inv2d) TRN2 Bass kernel — 8-core data-parallel over batch.

Math (per reference):
  Wr = w_reduce @ X          (1x1 conv, per pixel)         [b_reduce dropped:
                                                            training-mode BN is
                                                            shift-invariant]
  Wn = relu(gamma * (Wr - mean)/sqrt(var+eps) + beta)      (batch stats over B,H,W
                                                            -> tiny AllReduce)
  Ker = w_span @ Wn + b_span                               (1x1 conv, C->C*9)
  out[c,p] = sum_k patches[c,k,p] * Ker[9c+k,p]            (3x3 involution)

The end-to-end wall time is dominated by the axon tunnel (~45 MB/s), so
the transfer format is aggressively shrunk:
  - X travels as int8 with per-(sample,channel) scales, dequantized on
    device into bf16; out travels as int8 + per-(sample,channel) scales
    (computed on device), dequantized on host.
  - The big weights travel once (1/8 shard per core) and are AllGathered
    on device over NeuronLink.
  - Compute is bf16 with fp32 PSUM/stat accumulation.
  - The runner invokes the bass_exec custom call directly with no donated
    zero output buffers (the kernel writes every output element).
"""

import numpy as np

import concourse.bacc as bacc
import concourse.mybir as mybir
import concourse.tile as tile

F32 = mybir.dt.float32
BF16 = mybir.dt.bfloat16
I8 = mybir.dt.int8
AF = mybir.ActivationFunctionType
ALU = mybir.AluOpType

B, C, H, W = 16, 256, 64, 64
K2 = 9
NCORES = 8
BL = B // NCORES           # samples per core
HW = H * W
NP = 128                   # partitions
NCH = C // NP              # 2 channel chunks of 128
PB = 8                     # pixel blocks per sample
PBS = HW // PB             # 512 pixels per block
PH = H // PB               # 8 image rows per block
EPS = 1e-5
NTOT = float(B * HW)
PW = W + 2                 # 66 padded width
WSH = NP // NCORES         # 16 weight rows uploaded per core

_CACHE = {}


def _emit(ctx, nc, tc, X, xsc_d, w_rT_s, w_spT_s, b_sp_d, gamma_d, beta_d,
          outp):
    # single fused output: int8 data followed by the f32 scales as raw bytes
    # (a separate small output tensor would cost an extra ~80 ms tunnel RTT)
    out = outp[0:BL * C * HW].rearrange("(s c h w) -> s c h w", s=BL, c=C, h=H)
    osc_d = outp[BL * C * HW:].rearrange("(p b) -> p b", p=NP)  # int8 bytes
    pp = ctx.enter_context(tc.tile_pool(name="persist", bufs=1))
    junkp = ctx.enter_context(tc.tile_pool(name="junk", bufs=2))
    psA = ctx.enter_context(tc.tile_pool(name="psA", bufs=2, space="PSUM"))
    psS = ctx.enter_context(tc.tile_pool(name="psS", bufs=5, space="PSUM"))
    dramp = ctx.enter_context(tc.tile_pool(name="drambp", bufs=1, space="DRAM"))

    # ---- persistent tiles ----
    w_rT = pp.tile([NP, NCH, C], BF16)           # [cin, kc, cout]
    w_spT = pp.tile([NP, NCH, K2, C], BF16)      # [cin, kc, k, cout]
    b_spv = pp.tile([NP, NCH, K2], F32)          # b_span[9c+k] -> [c, ch, k]
    gam = pp.tile([NP, NCH], F32)
    bet = pp.tile([NP, NCH], F32)
    xq = pp.tile([NP, BL, NCH, H, W], I8)        # quantized X staging
    xsc = pp.tile([NP, BL, NCH, PB], F32)        # X dequant scales (per 8-row blk)
    xpad = pp.tile([NP, BL, NCH, H + 2, PW], BF16)
    wr = pp.tile([NP, BL, NCH, HW], BF16)        # Wr, normalized in place -> Wn
    obuf = pp.tile([NP, BL, NCH, PB, PBS], BF16)  # involution result
    oq = pp.tile([NP, BL, NCH, HW], I8)          # quantized out staging
    oamax = pp.tile([NP, BL, NCH, PB], F32)
    orinv = pp.tile([NP, BL, NCH, PB], F32)
    osc = pp.tile([NP, BL, NCH, PB], F32)
    mean_parts = pp.tile([NP, NCH, BL * PB], F32)
    sq_parts = pp.tile([NP, NCH, BL * PB], F32)
    cc_sb = pp.tile([NP, 2 * NCH], F32)
    stats = pp.tile([NP, 2 * NCH], F32)
    mean_t = pp.tile([NP, NCH], F32)
    var_t = pp.tile([NP, NCH], F32)
    tmp_a = pp.tile([NP, NCH], F32)
    tmp_b = pp.tile([NP, NCH], F32)
    rinv = pp.tile([NP, NCH], F32)
    scale_bn = pp.tile([NP, NCH], F32)
    shift_bn = pp.tile([NP, NCH], F32)

    cc_in = dramp.tile([NP, 2 * NCH], F32)
    cc_out = dramp.tile([NP, 2 * NCH], F32)
    wsh_r = dramp.tile([WSH, NCH, C], BF16)
    wsh_sp = dramp.tile([WSH, NCH, K2, C], BF16)
    wg_r = dramp.tile([NP, NCH, C], BF16)
    wg_sp = dramp.tile([NP, NCH, K2, C], BF16)

    groups = [list(range(NCORES))]

    # ---- weights: AllGather the per-core shards, then load to SBUF ----
    # (collectives can't read IO tensors: stage via DRAM scratch first)
    nc.sync.dma_start(wsh_r, w_rT_s)
    nc.sync.dma_start(wsh_sp, w_spT_s)
    nc.gpsimd.collective_compute(
        "AllGather", ALU.bypass, replica_groups=groups,
        ins=[wsh_r.opt()], outs=[wg_r.opt()],
    )
    nc.gpsimd.collective_compute(
        "AllGather", ALU.bypass, replica_groups=groups,
        ins=[wsh_sp.opt()], outs=[wg_sp.opt()],
    )
    nc.sync.dma_start(w_rT, wg_r)
    nc.sync.dma_start(w_spT, wg_sp)
    nc.sync.dma_start(b_spv, b_sp_d)
    nc.sync.dma_start(gam, gamma_d)
    nc.sync.dma_start(bet, beta_d)
    nc.sync.dma_start(xsc, xsc_d)

    # ---- X: int8 in, dequantize to bf16 into the padded tile ----
    for s in range(BL):
        for ch in range(NCH):
            nc.vector.memset(xpad[:, s, ch, 0, :], 0.0)
            nc.vector.memset(xpad[:, s, ch, H + 1, :], 0.0)
            nc.vector.memset(xpad[:, s, ch, 1:H + 1, 0:1], 0.0)
            nc.vector.memset(xpad[:, s, ch, 1:H + 1, W + 1:W + 2], 0.0)
            nc.sync.dma_start(xq[:, s, ch], X[s, ch * NP:(ch + 1) * NP, :, :])
            for pb in range(PB):
                nc.scalar.activation(
                    xpad[:, s, ch, 1 + pb * PH:1 + (pb + 1) * PH, 1:W + 1],
                    xq[:, s, ch, pb * PH:(pb + 1) * PH, :], AF.Copy,
                    scale=xsc[:, s, ch, pb:pb + 1])

    prodsp = ctx.enter_context(tc.tile_pool(name="prods", bufs=1))

    # ---- phase A: Wr = w_reduce @ X, with stats partials ----
    for s in range(BL):
        for ch in range(NCH):
            for pb in range(PB):
                ps = psA.tile([NP, PBS], F32, name="psa")
                for kc in range(NCH):
                    rhs = xpad[:, s, kc, 1 + pb * PH:1 + (pb + 1) * PH, 1:W + 1]
                    nc.tensor.matmul(
                        ps,
                        lhsT=w_rT[:, kc, ch * NP:(ch + 1) * NP],
                        rhs=rhs,
                        start=(kc == 0), stop=(kc == NCH - 1),
                    )
                idx = s * PB + pb
                nc.scalar.activation(
                    wr[:, s, ch, pb * PBS:(pb + 1) * PBS], ps, AF.Copy,
                    accum_out=mean_parts[:, ch, idx:idx + 1])
                junk = junkp.tile([NP, PBS], F32, name="junk")
                nc.scalar.activation(
                    junk, ps, AF.Square,
                    accum_out=sq_parts[:, ch, idx:idx + 1])

    # ---- BN stats: local partials -> AllReduce -> scale/shift ----
    for ch in range(NCH):
        nc.vector.reduce_sum(cc_sb[:, ch:ch + 1], mean_parts[:, ch, :],
                             axis=mybir.AxisListType.X)
        nc.vector.reduce_sum(cc_sb[:, NCH + ch:NCH + ch + 1], sq_parts[:, ch, :],
                             axis=mybir.AxisListType.X)
    nc.sync.dma_start(cc_in, cc_sb)
    nc.gpsimd.collective_compute(
        "AllReduce", ALU.add,
        replica_groups=groups,
        ins=[cc_in.opt()], outs=[cc_out.opt()],
    )
    nc.sync.dma_start(stats, cc_out)

    nc.vector.tensor_scalar_mul(mean_t, stats[:, 0:NCH], 1.0 / NTOT)
    nc.vector.tensor_scalar_mul(var_t, stats[:, NCH:2 * NCH], 1.0 / NTOT)
    nc.vector.tensor_tensor(tmp_a, mean_t, mean_t, op=ALU.mult)
    nc.vector.tensor_tensor(var_t, var_t, tmp_a, op=ALU.subtract)
    nc.vector.tensor_scalar_add(var_t, var_t, EPS)
    # rsqrt: ACT Sqrt of DVE reciprocal, then 2 Newton steps (x *= 1.5 - 0.5*v*x^2)
    nc.vector.reciprocal(rinv, var_t)
    nc.scalar.sqrt(rinv, rinv)
    for _ in range(2):
        nc.vector.tensor_tensor(tmp_a, rinv, rinv, op=ALU.mult)
        nc.vector.tensor_tensor(tmp_a, tmp_a, var_t, op=ALU.mult)
        nc.vector.tensor_scalar(tmp_a, tmp_a, -0.5, 1.5, op0=ALU.mult, op1=ALU.add)
        nc.vector.tensor_tensor(rinv, rinv, tmp_a, op=ALU.mult)
    nc.vector.tensor_tensor(scale_bn, rinv, gam, op=ALU.mult)
    nc.vector.tensor_tensor(tmp_b, mean_t, scale_bn, op=ALU.mult)
    nc.vector.tensor_tensor(shift_bn, bet, tmp_b, op=ALU.subtract)

    # ---- normalize+ReLU in place: wr -> Wn ----
    for s in range(BL):
        for ch in range(NCH):
            nc.scalar.activation(wr[:, s, ch, :], wr[:, s, ch, :], AF.Relu,
                                 scale=scale_bn[:, ch:ch + 1],
                                 bias=shift_bn[:, ch:ch + 1])

    # ---- span matmul + involution ----
    for s in range(BL):
        for pb in range(PB):
            for ch in range(NCH):
                prods = prodsp.tile([NP, K2, PBS], F32, name="prods")
                for k in range(K2):
                    ps2 = psS.tile([NP, PBS], F32, name="pss")
                    for kc in range(NCH):
                        nc.tensor.matmul(
                            ps2,
                            lhsT=w_spT[:, kc, k, ch * NP:(ch + 1) * NP],
                            rhs=wr[:, s, kc, pb * PBS:(pb + 1) * PBS],
                            start=(kc == 0), stop=(kc == NCH - 1),
                        )
                    di, dj = k // 3, k % 3
                    patch = xpad[:, s, ch, di + pb * PH:di + (pb + 1) * PH, dj:dj + W]
                    nc.vector.scalar_tensor_tensor(
                        out=prods[:, k, :].rearrange("p (h w) -> p h w", h=PH),
                        in0=ps2.rearrange("p (h w) -> p h w", h=PH),
                        scalar=b_spv[:, ch, k:k + 1],
                        in1=patch,
                        op0=ALU.add, op1=ALU.mult,
                    )
                # DVE reduce accumulates fp32 internally; only the final
                # write is rounded to bf16.
                with nc.allow_low_precision(reason="bf16 output of 9-term sum"):
                    nc.vector.reduce_sum(obuf[:, s, ch, pb, :],
                                         prods.rearrange("p k f -> p f k"),
                                         axis=mybir.AxisListType.X)

    # ---- quantize out to int8 with per-(sample,channel,8-row-blk) scales ----
    for s in range(BL):
        for ch in range(NCH):
            nc.vector.tensor_reduce(oamax[:, s, ch, :],
                                    obuf[:, s, ch, :, :],
                                    op=ALU.max, axis=mybir.AxisListType.X,
                                    apply_absolute_value=True)
    nc.vector.tensor_scalar_add(oamax, oamax, 1e-30)
    nc.vector.reciprocal(orinv, oamax)
    nc.vector.tensor_scalar_mul(orinv, orinv, 127.0)
    nc.vector.tensor_scalar_mul(osc, oamax, 1.0 / 127.0)
    nc.sync.dma_start(osc_d, osc.rearrange("p a b c -> p (a b c)").bitcast(I8))
    for s in range(BL):
        for ch in range(NCH):
            for pb in range(PB):
                with nc.allow_low_precision(reason="int8 quantized output"):
                    nc.scalar.activation(
                        oq[:, s, ch, pb * PBS:(pb + 1) * PBS],
                        obuf[:, s, ch, pb, :],
                        AF.Copy, scale=orinv[:, s, ch, pb:pb + 1])
            nc.sync.dma_start(
                out[s, ch * NP:(ch + 1) * NP, :, :],
                oq[:, s, ch, :].rearrange("p (h w) -> p h w", h=H))


def _build():
    nc = bacc.Bacc("TRN2", target_bir_lowering=False, debug=False,
                   enable_asserts=False, num_devices=NCORES)
    X = nc.dram_tensor("X", [BL, C, H, W], I8, kind="ExternalInput").ap()
    xsc = nc.dram_tensor("xsc", [NP, BL, NCH, PB], F32,
                         kind="ExternalInput").ap()
    w_rT_s = nc.dram_tensor("w_rT_s", [WSH, NCH, C], BF16,
                            kind="ExternalInput").ap()
    w_spT_s = nc.dram_tensor("w_spT_s", [WSH, NCH, K2, C], BF16,
                             kind="ExternalInput").ap()
    b_spv = nc.dram_tensor("b_spv", [NP, NCH, K2], F32, kind="ExternalInput").ap()
    gamma = nc.dram_tensor("gamma2", [NP, NCH], F32, kind="ExternalInput").ap()
    beta = nc.dram_tensor("beta2", [NP, NCH], F32, kind="ExternalInput").ap()
    outp = nc.dram_tensor("outp", [BL * C * HW + NP * BL * NCH * PB * 4], I8,
                          kind="ExternalOutput").ap()

    from contextlib import ExitStack

    with tile.TileContext(nc) as tc:
        with ExitStack() as ctx:
            _emit(ctx, nc, tc, X, xsc, w_rT_s, w_spT_s, b_spv, gamma, beta,
                  outp)
    nc.compile()
    return nc


def get_nc():
    if "nc" not in _CACHE:
        _CACHE["nc"] = _build()
    return _CACHE["nc"]


def _prep_weights(inputs: dict) -> dict:
    """Cast + rearrange the (small) weight inputs into per-core layouts."""
    import ml_dtypes

    bf16 = ml_dtypes.bfloat16
    w_reduce = np.asarray(inputs["w_reduce"], dtype=np.float32)
    w_span = np.asarray(inputs["w_span"], dtype=np.float32)
    b_span = np.asarray(inputs["b_span"], dtype=np.float32)
    gamma = np.asarray(inputs["gamma"], dtype=np.float32)
    beta = np.asarray(inputs["beta"], dtype=np.float32)

    # w_rT[p, kc, o] = w_reduce[o, kc*NP + p]; upload 1/8 shard per core
    w_rT = np.ascontiguousarray(
        w_reduce.T.reshape(NCH, NP, C).transpose(1, 0, 2)).astype(bf16)
    # w_spT[p, kc, k, co] = w_span[9*co + k, kc*NP + p]
    w_spT = np.ascontiguousarray(
        w_span.reshape(C, K2, C).transpose(2, 1, 0)
        .reshape(NCH, NP, K2, C).transpose(1, 0, 2, 3)).astype(bf16)
    # b_spv[p, ch, k] = b_span[9*(ch*NP+p) + k]
    b_spv = np.ascontiguousarray(
        b_span.reshape(NCH, NP, K2).transpose(1, 0, 2))
    gam = np.ascontiguousarray(gamma.reshape(NCH, NP).T)
    bet = np.ascontiguousarray(beta.reshape(NCH, NP).T)

    return {
        "w_rT_s": w_rT,      # [128, ...] == concat of 8 x [16, ...] shards
        "w_spT_s": w_spT,
        "b_spv": np.tile(b_spv, (NCORES, 1, 1)),
        "gamma2": np.tile(gam, (NCORES, 1)),
        "beta2": np.tile(bet, (NCORES, 1)),
    }


def _quant_X(X: np.ndarray):
    """int8-quantize X with per-(sample,channel,8-row-block) scales.

    Chunked over samples on a thread pool — numpy releases the GIL for the
    large ufuncs, so this scales with cores.
    """
    from concurrent.futures import ThreadPoolExecutor

    Xb = X.reshape(B, C, PB, PH * W)
    amax = np.empty((B, C, PB), np.float32)
    Xq = np.empty((B, C, PB, PH * W), np.int8)

    def quant_sample(b):
        a = np.abs(Xb[b]).max(axis=2)
        np.maximum(a, 1e-30, out=a)
        amax[b] = a
        tmp = Xb[b] * (127.0 / a)[:, :, None]
        np.rint(tmp, out=tmp)
        Xq[b] = tmp.astype(np.int8)

    with ThreadPoolExecutor(8) as pool:
        list(pool.map(quant_sample, range(B)))

    # xsc[p, s_local, ch, pb] for core i covers sample s = i*BL + s_local,
    # channel c = ch*NP + p; concat over cores on axis 0.
    scale = (amax / 127.0).reshape(NCORES, BL, NCH, NP, PB)          # [i,s,ch,p,pb]
    xsc = np.ascontiguousarray(scale.transpose(0, 3, 1, 2, 4)).reshape(
        NCORES * NP, BL, NCH, PB)
    return Xq.reshape(B, C, H, W), xsc


def _prep_host(inputs: dict) -> dict:
    """Full host prep (used by the trace path)."""
    Xq, xsc = _quant_X(np.asarray(inputs["X"], dtype=np.float32))
    return {"X": Xq, "xsc": xsc, **_prep_weights(inputs)}


def _unprep_host(out_q: np.ndarray, osc: np.ndarray) -> np.ndarray:
    """Dequantize int8 output with per-(sample,channel,block) scales -> f32."""
    # osc concat: (NCORES*NP, BL, NCH, PB); sample s = i*BL+s_l, c = ch*NP+p
    sc = osc.reshape(NCORES, NP, BL, NCH, PB).transpose(0, 2, 3, 1, 4).reshape(
        B, C, PB)
    outf = out_q.astype(np.float32).reshape(B, C, PB, PH * W)
    outf *= sc[:, :, :, None]
    return outf.reshape(B, C, H, W)


_PCD = BL * C * HW                       # per-core int8 data bytes
_PCS = NP * BL * NCH * PB * 4            # per-core scale bytes (f32)


def _decode_outp(raw: np.ndarray) -> np.ndarray:
    """Split the fused int8 output into data + scales and dequantize."""
    from concurrent.futures import ThreadPoolExecutor

    percore = raw.reshape(NCORES, _PCD + _PCS)
    # scales: [p, (s, ch, pb)] f32 bytes per core
    osc = percore[:, _PCD:].reshape(NCORES * NP, 4 * BL * NCH * PB).view(
        np.float32).reshape(NCORES * NP, BL, NCH, PB)
    sc = osc.reshape(NCORES, NP, BL, NCH, PB).transpose(0, 2, 3, 1, 4).reshape(
        B, C, PB)
    out_b = percore[:, :_PCD].reshape(B, C, PB, PH * W)

    full = np.empty((B, C, H, W), np.float32)
    fullb = full.reshape(B, C, PB, PH * W)

    def deq(b):
        fb = out_b[b].astype(np.float32)
        fb *= sc[b][:, :, None]
        fullb[b] = fb

    with ThreadPoolExecutor(8) as pool:
        list(pool.map(deq, range(B)))
    return full


def _get_exec():
    """Build (once) the jitted shard_map executor around the bass_exec call."""
    if "exec" in _CACHE:
        return _CACHE["exec"]

    import jax
    from jax.sharding import Mesh, PartitionSpec
    from jax.experimental.shard_map import shard_map
    from concourse.bass2jax import (_bass_exec_p, install_neuronx_cc_hook,
                                    partition_id_tensor)

    nc = get_nc()
    install_neuronx_cc_hook()

    partition_name = (nc.partition_id_tensor.name
                      if nc.partition_id_tensor else None)
    in_names, out_names, out_avals = [], [], []
    for alloc in nc.m.functions[0].allocations:
        if not isinstance(alloc, mybir.MemoryLocationSet):
            continue
        name = alloc.memorylocations[0].name
        if alloc.kind == "ExternalInput":
            if name != partition_name:
                in_names.append(name)
        elif alloc.kind == "ExternalOutput":
            out_names.append(name)
            out_avals.append(jax.core.ShapedArray(
                tuple(alloc.tensor_shape), mybir.dt.np(alloc.dtype)))
    in_names_all = list(in_names)
    if partition_name is not None:
        in_names_all.append(partition_name)

    def _body(*args):
        operands = list(args)
        if partition_name is not None:
            operands.append(partition_id_tensor())
        outs = _bass_exec_p.bind(
            *operands,
            out_avals=tuple(out_avals),
            in_names=tuple(in_names_all),
            out_names=tuple(out_names),
            lowering_input_output_aliases=(),
            sim_require_finite=True,
            sim_require_nnan=True,
            nc=nc,
        )
        return tuple(outs)

    devices = jax.devices()[:NCORES]
    mesh = Mesh(np.asarray(devices), ("core",))
    sharded = jax.jit(
        shard_map(_body, mesh=mesh,
                  in_specs=(PartitionSpec("core"),) * len(in_names),
                  out_specs=(PartitionSpec("core"),) * len(out_names),
                  check_rep=False),
        keep_unused=True,
    )
    from jax.sharding import NamedSharding

    _CACHE["exec"] = (sharded, in_names, out_names,
                      NamedSharding(mesh, PartitionSpec("core")))
    return _CACHE["exec"]


def run(inputs: dict, trace: bool = False):
    """Run on 8 cores; returns (full_output_f32, results_shim)."""
    if trace:
        # profiling path through run_bass_kernel_spmd (NTFF capture)
        from concourse.bass_utils import run_bass_kernel_spmd

        prep = _prep_host(inputs)
        nc = get_nc()
        in_maps = []
        for c in range(NCORES):
            m = {}
            for k, v in prep.items():
                n = v.shape[0] // NCORES
                m[k] = np.ascontiguousarray(v[c * n:(c + 1) * n])
            in_maps.append(m)
        res = run_bass_kernel_spmd(nc, in_maps, list(range(NCORES)), trace=True)
        raw = np.concatenate([r["outp"] for r in res.results], axis=0)
        return _decode_outp(raw), res

    import jax
    from concurrent.futures import ThreadPoolExecutor

    sharded, in_names, out_names, nsh = _get_exec()
    devices = list(nsh.mesh.devices.ravel())

    # weights are small and independent of X: start their upload first
    dev_args = {k: jax.device_put(v, nsh)
                for k, v in _prep_weights(inputs).items()}

    # stream X: quantize one core-shard at a time and start its upload
    # immediately, so host quantization hides under the tunnel transfer
    X = np.asarray(inputs["X"], dtype=np.float32)
    Xb = X.reshape(B, C, PB, PH * W)
    amax = np.empty((B, C, PB), np.float32)
    x_shards = []

    def quant_sample(b):
        a = np.abs(Xb[b]).max(axis=2)
        np.maximum(a, 1e-30, out=a)
        amax[b] = a
        tmp = Xb[b] * (127.0 / a)[:, :, None]
        np.rint(tmp, out=tmp)
        return tmp.astype(np.int8).reshape(C, H, W)

    with ThreadPoolExecutor(2) as pool:
        for c in range(NCORES):
            qs = list(pool.map(quant_sample, range(c * BL, (c + 1) * BL)))
            x_shards.append(jax.device_put(np.stack(qs), devices[c]))
    dev_args["X"] = jax.make_array_from_single_device_arrays(
        (B, C, H, W), nsh, x_shards)

    scale = (amax / 127.0).reshape(NCORES, BL, NCH, NP, PB)          # [i,s,ch,p,pb]
    xsc = np.ascontiguousarray(scale.transpose(0, 3, 1, 2, 4)).reshape(
        NCORES * NP, BL, NCH, PB)
    dev_args["xsc"] = jax.device_put(xsc, nsh)

    outs = sharded(*[dev_args[name] for name in in_names])
    outs[0].copy_to_host_async()           # pre-arm D2H so it starts at exec end
    raw = np.asarray(outs[0])              # (NCORES * percore,) int8
    full = _decode_outp(raw)

    class _Res:
        exec_time_ns = None
        mean_exec_time_ns = None

    return full, _Res()


def kernel(**inputs) -> np.ndarray:
    full, _ = run(inputs, trace=False)
    return full
